# revision 2
# baseline (speedup 1.0000x reference)
"""NNUE HalfKP embedding-bag kernel for 8x Trainium2 NeuronCores.

Reference computation:
    stm_ft  = einsum('bk,bkf->bf', values, ft_w[stm_idx])  + ft_b
    nstm_ft = einsum('bk,bkf->bf', values, ft_w[nstm_idx]) + ft_b
    v_stm   = einsum('bk,bkf->bf', values, fft_w[stm_idx  % 640]) + fft_b
    v_nstm  = einsum('bk,bkf->bf', values, fft_w[nstm_idx % 640]) + fft_b
    hidden  = clip(concat([stm_ft + v_stm, nstm_ft + v_nstm]), 0, 1)   # [B, 1024]
    l1      = hidden @ out_w.T + out_b                                  # [B, 8]
    out     = sigmoid(l1[b, buckets[b]])                                # [B, 1]

Design (batch data-parallel: 1024 rows per core, table replicated):
  * ft_w[i] + fft_w[i % 640] share the per-(b,k) weight, so gather from ONE
    combined fp8-e4m3 table T[v] = ft_w[v] + fft_w[v % 640] (21 MB, 4x less
    traffic than two f32 gathers; fp8 end-to-end rel_fro ~ 1e-3 vs 2e-2 gate).
  * The hard bottleneck is gpsimd.dma_gather descriptor generation on the Q7
    DSP: measured ~7.9 ns per index, flat in elem_size/dtype, one SWDGE queue,
    and the ucode walks roundup(num_idxs, 128) regardless of runtime count or
    trailing negative indices.  Hence: exactly 4096 walked indices per
    128-row chunk x index-set, zero padding.
      - int16 gather indices cap the view at 32768 rows, so lookups split
        into a LOW list (T[0:32768]) and HIGH list (T[8193:40961]).  Entries
        with 8193 <= v < 32768 are assigned to whichever list balances both
        to exactly 2048 (strict classes are ~819 +- 27 << 2048, so the
        balance never fails for any input).
      - The count registers are hoisted via to_reg() outside the loop: the
        previous per-gather constants emitted MOVEs whose register-reuse
        hazard made each gather wait on the prior gather's DMA completion
        (that stall was ~40% of the v1 runtime).  Gathers now run
        back-to-back (~46 ns gaps), Q7-bound at its floor.
  * Weighted sums run on the PE from host-prebuilt fp8 routing matrices
    (lhsT[slot, m] = values[m, k], duplicates merged implicitly): 16
    DoubleRow fp8 matmuls per chunk-set accumulate 32 runs in PSUM.
  * ft_b + fft_b is added by one extra static matmul (lhsT column 0 = ones,
    rhs partition 0 = fp8 bias row) instead of wasting a gathered run.
  * clip -> bf16 hidden; bucket selection folded host-side into
    w_sel = out_w[buckets]; logit = reduce(hidden * w_sel) + out_b[buckets];
    sigmoid on the scalar engine.
"""

import sys

sys.path.insert(0, "/opt/trn_rl_repo")

import numpy as np
import ml_dtypes

import concourse.mybir as mybir
from concourse import bacc
from concourse.tile import TileContext
from concourse.bass_utils import run_bass_kernel_spmd

BF16 = ml_dtypes.bfloat16
FP8 = ml_dtypes.float8_e4m3

B = 8192
K = 32
F = 512
FT_VOCAB = 40960
FFT_VOCAB = 640
N_CORES = 8
BC = B // N_CORES          # rows per core = 1024
CH = BC // 128             # chunks per core = 8
HSTART = 8193              # high table = T[HSTART : HSTART + 32768]
LOW_CNT = 2048             # exactly filled via balanced assignment
LOW_RUNS = LOW_CNT // 128  # 16
HIGH_CNT = 2048            # exactly filled (bias handled by a static matmul)
HIGH_RUNS = HIGH_CNT // 128  # 16
RUNS = LOW_RUNS + HIGH_RUNS  # 32; Q7 walks exactly 4096 indices per chunk-set
NIDX16 = (LOW_CNT + HIGH_CNT) // 16   # 256

_compiled = None


def _build():
    nc = bacc.Bacc("TRN2", target_bir_lowering=False, debug=False, num_devices=N_CORES)

    t_d = nc.dram_tensor("t_tab", [FT_VOCAB + 1, F], mybir.dt.float8e4, kind="ExternalInput")
    # idx laid out exactly as the SBUF tile: [128, CH * 2 * NIDX16]
    idx_d = nc.dram_tensor("idx", [128, CH * 2 * NIDX16], mybir.dt.int16, kind="ExternalInput")
    # W per chunk holds both sets: [CH, 128, 2 * RUNS * 128] fp8
    w_d = nc.dram_tensor("w_rt", [CH, 128, 2 * RUNS * 128], mybir.dt.float8e4, kind="ExternalInput")
    wsel_d = nc.dram_tensor("w_sel", [CH, 128, 2 * F], mybir.dt.bfloat16, kind="ExternalInput")
    # bias via one static matmul: lhsT=w32 (col p=0 all-ones), rhs=bias_row
    # (partition 0 = fp8 bias row) -> psum[m, f] += bias[f]
    brow_d = nc.dram_tensor("bias_row", [128, F], mybir.dt.float8e4, kind="ExternalInput")
    w32_d = nc.dram_tensor("w32", [128, 128], mybir.dt.float8e4, kind="ExternalInput")
    obsel_d = nc.dram_tensor("ob_sel", [CH, 128, 1], mybir.dt.float32, kind="ExternalInput")
    out_d = nc.dram_tensor("out", [BC, 1], mybir.dt.float32, kind="ExternalOutput")

    t_low = t_d[:32768, :]
    t_high = t_d[HSTART : HSTART + 32768, :]

    with TileContext(nc) as tc:
        with (
            tc.tile_pool(name="idxp", bufs=1) as idxp,
            tc.tile_pool(name="gath", bufs=4) as gathp,
            tc.tile_pool(name="wblk", bufs=3) as wblkp,
            tc.tile_pool(name="psum", bufs=4, space="PSUM") as psump,
            tc.tile_pool(name="hid", bufs=2) as hidp,
            tc.tile_pool(name="wsel", bufs=2) as wselp,
            tc.tile_pool(name="fin", bufs=4) as finp,
        ):
            idxt = idxp.tile([128, CH * 2 * NIDX16], mybir.dt.int16)
            for ch in range(CH):
                nc.sync.dma_start(
                    out=idxt[:, ch * 2 * NIDX16 : (ch + 1) * 2 * NIDX16],
                    in_=idx_d[:, ch * 2 * NIDX16 : (ch + 1) * 2 * NIDX16],
                )
            browt = idxp.tile([128, F], mybir.dt.float8e4)
            nc.sync.dma_start(out=browt[:], in_=brow_d[:, :])
            w32t = idxp.tile([128, 128], mybir.dt.float8e4)
            nc.sync.dma_start(out=w32t[:], in_=w32_d[:, :])

            low_reg = nc.gpsimd.to_reg(LOW_CNT)
            high_reg = nc.gpsimd.to_reg(HIGH_CNT)

            for ch in range(CH):
                wblk = wblkp.tile([128, 2 * RUNS * 128], mybir.dt.float8e4, tag="wblk")
                nc.sync.dma_start(out=wblk[:], in_=w_d[ch])

                hid = hidp.tile([128, 2 * F], mybir.dt.bfloat16)

                for s in range(2):
                    ibase = (ch * 2 + s) * NIDX16
                    rt = gathp.tile([128, RUNS * F], mybir.dt.float8e4, tag="gath")
                    nc.gpsimd.dma_gather(
                        out_ap=rt[:, : LOW_RUNS * F].rearrange("p (s f) -> p s f", f=F),
                        in_ap=t_low,
                        idxs_ap=idxt[:, ibase : ibase + LOW_CNT // 16],
                        num_idxs=LOW_CNT,
                        num_idxs_reg=low_reg,
                        elem_size=F,
                        single_packet=False,
                    )
                    nc.gpsimd.dma_gather(
                        out_ap=rt[:, LOW_RUNS * F :].rearrange("p (s f) -> p s f", f=F),
                        in_ap=t_high,
                        idxs_ap=idxt[:, ibase + LOW_CNT // 16 : ibase + NIDX16],
                        num_idxs=HIGH_CNT,
                        num_idxs_reg=high_reg,
                        elem_size=F,
                        single_packet=False,
                    )

                    ps = psump.tile([128, F], mybir.dt.float32, tag="ps")
                    for q in range(RUNS // 2):
                        wq = wblk[:, (s * RUNS + 2 * q) * 128 : (s * RUNS + 2 * q + 2) * 128]
                        nc.tensor.matmul(
                            out=ps[:],
                            lhsT=wq.rearrange("p (two f) -> p two f", two=2),
                            rhs=rt[:, 2 * q * F : (2 * q + 2) * F].rearrange(
                                "p (two f) -> p two f", two=2
                            ),
                            start=(q == 0),
                            stop=False,
                            perf_mode=mybir.MatmulPerfMode.DoubleRow,
                        )
                    nc.tensor.matmul(
                        out=ps[:],
                        lhsT=w32t[:],
                        rhs=browt[:],
                        start=False,
                        stop=True,
                    )

                    nc.vector.tensor_scalar(
                        out=hid[:, s * F : (s + 1) * F],
                        in0=ps[:],
                        scalar1=0.0,
                        scalar2=1.0,
                        op0=mybir.AluOpType.max,
                        op1=mybir.AluOpType.min,
                    )

                wsel = wselp.tile([128, 2 * F], mybir.dt.bfloat16)
                nc.sync.dma_start(out=wsel[:], in_=wsel_d[ch])
                obsel = finp.tile([128, 1], mybir.dt.float32, tag="ob")
                nc.sync.dma_start(out=obsel[:], in_=obsel_d[ch])

                prod = finp.tile([128, 2 * F], mybir.dt.float32, tag="prod")
                nc.vector.tensor_tensor(
                    out=prod[:], in0=hid[:], in1=wsel[:], op=mybir.AluOpType.mult
                )
                acc = finp.tile([128, 1], mybir.dt.float32, tag="acc")
                nc.vector.tensor_reduce(
                    out=acc[:], in_=prod[:], axis=mybir.AxisListType.X, op=mybir.AluOpType.add
                )
                sig = finp.tile([128, 1], mybir.dt.float32, tag="sig")
                nc.scalar.activation(
                    out=sig[:],
                    in_=acc[:],
                    func=mybir.ActivationFunctionType.Sigmoid,
                    bias=obsel[:],
                )
                nc.sync.dma_start(out=out_d[ch * 128 : (ch + 1) * 128, :], in_=sig[:])

    nc.compile()
    return nc


def _get_compiled():
    global _compiled
    if _compiled is None:
        _compiled = _build()
    return _compiled


def _wrap16(lst):
    """int16 index list -> [128, len/16] wrapped (entry j -> [j%16, j//16]),
    replicated to all 128 partitions."""
    n = lst.shape[0]
    w = lst.reshape(n // 16, 16).T.astype(np.int16)
    return np.tile(w, (8, 1))


def _prep_set(idx_core, values_core):
    """Per-core prep for one index set.

    idx_core: [BC, K] int32, values_core: [BC, K] f32
    Returns idx16 [CH, 128, NIDX16] int16, W [CH, 128, RUNS*128] f32.
    """
    idx16 = np.zeros((CH, 128, NIDX16), np.int16)
    W = np.zeros((CH, 128, RUNS, 128), np.float32)
    for ch in range(CH):
        v = idx_core[ch * 128 : (ch + 1) * 128].reshape(-1)        # [4096]
        val = values_core[ch * 128 : (ch + 1) * 128].reshape(-1)
        m_of = np.repeat(np.arange(128), K)

        strict_low = v < HSTART
        flex = (~strict_low) & (v < 32768)
        need = LOW_CNT - int(strict_low.sum())
        assert 0 <= need <= int(flex.sum()), (need, int(flex.sum()))
        flex_rank = np.cumsum(flex) - 1
        is_low = strict_low | (flex & (flex_rank < need))
        lows = np.nonzero(is_low)[0]
        highs = np.nonzero(~is_low)[0]
        assert lows.shape[0] == LOW_CNT and highs.shape[0] == 4096 - LOW_CNT

        nh = highs.shape[0]
        assert nh == HIGH_CNT
        ilist = np.empty(LOW_CNT + HIGH_CNT, np.int16)
        ilist[:LOW_CNT] = v[lows]
        ilist[LOW_CNT:] = v[highs] - HSTART
        idx16[ch, :, : LOW_CNT // 16] = _wrap16(ilist[:LOW_CNT])
        idx16[ch, :, LOW_CNT // 16 :] = _wrap16(ilist[LOW_CNT:])

        pos = np.arange(LOW_CNT + HIGH_CNT)
        ent = np.concatenate([lows, highs])
        W[ch, pos % 128, pos // 128, m_of[ent]] = val[ent]
    return idx16, W.reshape(CH, 128, RUNS * 128)


def _prep_core(core, T8, values, stm, nstm, wsel_all, obsel_all):
    rows = slice(core * BC, (core + 1) * BC)
    v_core = values[rows]
    idx_stm, w_stm = _prep_set(stm[rows], v_core)
    idx_nstm, w_nstm = _prep_set(nstm[rows], v_core)

    # idx SBUF layout: [128, CH * 2 * NIDX16], chunk-major then set
    idx = np.empty((128, CH * 2 * NIDX16), np.int16)
    for ch in range(CH):
        idx[:, (ch * 2) * NIDX16 : (ch * 2 + 1) * NIDX16] = idx_stm[ch]
        idx[:, (ch * 2 + 1) * NIDX16 : (ch * 2 + 2) * NIDX16] = idx_nstm[ch]

    w_rt = np.concatenate([w_stm, w_nstm], axis=2).astype(FP8)  # [CH,128,2*RUNS*128]

    return {
        "t_tab": T8,
        "idx": idx,
        "w_rt": w_rt,
        "w_sel": wsel_all[rows].reshape(CH, 128, 2 * F).astype(BF16),
        "ob_sel": obsel_all[rows].reshape(CH, 128, 1).astype(np.float32),
    }


def build_in_maps(values, stm_indices, nstm_indices, buckets, ft_w, ft_b, fft_w, fft_b, out_w, out_b):
    values = np.asarray(values, dtype=np.float32)
    stm_indices = np.asarray(stm_indices, dtype=np.int32)
    nstm_indices = np.asarray(nstm_indices, dtype=np.int32)
    buckets = np.asarray(buckets, dtype=np.int32)
    ft_w = np.asarray(ft_w, dtype=np.float32)
    ft_b = np.asarray(ft_b, dtype=np.float32)
    fft_w = np.asarray(fft_w, dtype=np.float32)
    fft_b = np.asarray(fft_b, dtype=np.float32)
    out_w = np.asarray(out_w, dtype=np.float32)
    out_b = np.asarray(out_b, dtype=np.float32)

    T8 = np.concatenate([
        ft_w + np.tile(fft_w, (FT_VOCAB // FFT_VOCAB, 1)),
        np.zeros((1, F), np.float32),
    ]).astype(FP8)                                # [FT_VOCAB + 1, F]
    brow = np.zeros((128, F), np.float32)
    brow[0] = ft_b + fft_b
    brow = brow.astype(FP8)
    w32 = np.zeros((128, 128), np.float32)
    w32[0, :] = 1.0
    w32 = w32.astype(FP8)
    wsel_all = out_w[buckets]                     # [B, 1024] f32
    obsel_all = out_b[buckets]                    # [B] f32

    maps = [
        _prep_core(c, T8, values, stm_indices, nstm_indices, wsel_all, obsel_all)
        for c in range(N_CORES)
    ]
    for m in maps:
        m["bias_row"] = brow
        m["w32"] = w32
    return maps


def kernel(**inputs):
    nc = _get_compiled()
    in_maps = build_in_maps(**inputs)
    res = run_bass_kernel_spmd(nc, in_maps, core_ids=list(range(N_CORES)))
    out = np.concatenate([res.results[c]["out"] for c in range(N_CORES)], axis=0)
    return out.astype(np.float32)


# revision 3
# speedup vs baseline: 1.1867x; 1.1867x over previous
"""NNUE HalfKP embedding-bag kernel for 8x Trainium2 NeuronCores.

Reference computation:
    stm_ft  = einsum('bk,bkf->bf', values, ft_w[stm_idx])  + ft_b
    nstm_ft = einsum('bk,bkf->bf', values, ft_w[nstm_idx]) + ft_b
    v_stm   = einsum('bk,bkf->bf', values, fft_w[stm_idx  % 640]) + fft_b
    v_nstm  = einsum('bk,bkf->bf', values, fft_w[nstm_idx % 640]) + fft_b
    hidden  = clip(concat([stm_ft + v_stm, nstm_ft + v_nstm]), 0, 1)   # [B, 1024]
    l1      = hidden @ out_w.T + out_b                                  # [B, 8]
    out     = sigmoid(l1[b, buckets[b]])                                # [B, 1]

Design (batch data-parallel: 1024 rows per core, table replicated):
  * ft_w[i] + fft_w[i % 640] share the per-(b,k) weight, so gather from ONE
    combined fp8-e4m3 table T[v] = ft_w[v] + fft_w[v % 640] (21 MB; fp8
    end-to-end gives rel_fro ~ 1e-3 vs the 2e-2 gate).
  * The hard bottleneck is gpsimd.dma_gather descriptor generation on the Q7
    DSP: measured ~7.9 ns per index, flat in elem_size/dtype, one SWDGE
    queue, and the ucode walks roundup(num_idxs, 128) regardless of runtime
    count registers or trailing negative indices.  So the only lever is
    walked-index count:
      - rows are DEDUPED per 128-row chunk x index-set (4096 lookups hit
        ~3900 unique rows); duplicate lookups merge into the routing matrix
        (weights sum), shrinking the walk to 3968 = 31 runs of 128.
      - int16 gather indices cap a view at 32768 rows, so rows split into a
        LOW list (T[0:32768], exactly 1920) and HIGH list (T[8193:40961],
        the rest, 0-padded to 2048); rows with 8193 <= v < 32768 go to
        whichever list balances the fixed sizes.
      - count registers are hoisted via to_reg() outside the loop: per-gather
        constant MOVEs had a register-reuse hazard that made each gather wait
        on the prior gather's DMA completion (~40% of the v1 runtime).
        Gathers now run back-to-back (~46 ns gaps), Q7-bound at the floor.
  * Weighted sums on the PE from host-prebuilt fp8 routing matrices
    (lhsT[slot, m] = sum of values[m, k] over duplicate lookups): 15
    DoubleRow fp8 matmuls + 1 single + 1 static bias matmul per chunk-set
    accumulate in PSUM.  The ft_b + fft_b bias comes from the static matmul
    (lhsT column 0 = ones, rhs partition 0 = fp8 bias row), not a gather.
  * clip -> bf16 hidden; bucket selection folded host-side into
    w_sel = out_w[buckets]; logit = reduce(hidden * w_sel) + out_b[buckets];
    sigmoid on the scalar engine.
"""

import sys

sys.path.insert(0, "/opt/trn_rl_repo")

import numpy as np
import ml_dtypes

import concourse.mybir as mybir
from concourse import bacc
from concourse.tile import TileContext
from concourse.bass_utils import run_bass_kernel_spmd

BF16 = ml_dtypes.bfloat16
FP8 = ml_dtypes.float8_e4m3

B = 8192
K = 32
F = 512
FT_VOCAB = 40960
FFT_VOCAB = 640
N_CORES = 8
BC = B // N_CORES          # rows per core = 1024
CH = BC // 128             # chunks per core = 8
HSTART = 8193              # high table = T[HSTART : HSTART + 32768]
LOW_CNT = 1920             # deduped rows, balanced assignment fills exactly
LOW_RUNS = LOW_CNT // 128  # 15
HIGH_CNT = 2048            # deduped remainder (<= 2014 measured) + 0-pad
HIGH_RUNS = HIGH_CNT // 128  # 16
RUNS = LOW_RUNS + HIGH_RUNS  # 31; Q7 walks 3968 indices per chunk-set
NIDX16 = (LOW_CNT + HIGH_CNT) // 16   # 248

_compiled = None


def _build():
    nc = bacc.Bacc("TRN2", target_bir_lowering=False, debug=False, num_devices=N_CORES)

    t_d = nc.dram_tensor("t_tab", [FT_VOCAB + 1, F], mybir.dt.float8e4, kind="ExternalInput")
    # idx laid out exactly as the SBUF tile: [128, CH * 2 * NIDX16]
    idx_d = nc.dram_tensor("idx", [128, CH * 2 * NIDX16], mybir.dt.int16, kind="ExternalInput")
    # W per chunk holds both sets: [CH, 128, 2 * RUNS * 128] fp8
    w_d = nc.dram_tensor("w_rt", [CH, 128, 2 * RUNS * 128], mybir.dt.float8e4, kind="ExternalInput")
    wsel_d = nc.dram_tensor("w_sel", [CH, 128, 2 * F], mybir.dt.bfloat16, kind="ExternalInput")
    # bias via one static matmul: lhsT=w32 (col p=0 all-ones), rhs=bias_row
    # (partition 0 = fp8 bias row) -> psum[m, f] += bias[f]
    brow_d = nc.dram_tensor("bias_row", [128, F], mybir.dt.float8e4, kind="ExternalInput")
    w32_d = nc.dram_tensor("w32", [128, 128], mybir.dt.float8e4, kind="ExternalInput")
    obsel_d = nc.dram_tensor("ob_sel", [CH, 128, 1], mybir.dt.float32, kind="ExternalInput")
    out_d = nc.dram_tensor("out", [BC, 1], mybir.dt.float32, kind="ExternalOutput")

    t_low = t_d[:32768, :]
    t_high = t_d[HSTART : HSTART + 32768, :]

    with TileContext(nc) as tc:
        with (
            tc.tile_pool(name="idxp", bufs=1) as idxp,
            tc.tile_pool(name="gath", bufs=4) as gathp,
            tc.tile_pool(name="wblk", bufs=3) as wblkp,
            tc.tile_pool(name="psum", bufs=4, space="PSUM") as psump,
            tc.tile_pool(name="hid", bufs=2) as hidp,
            tc.tile_pool(name="wsel", bufs=2) as wselp,
            tc.tile_pool(name="fin", bufs=4) as finp,
        ):
            idxt = idxp.tile([128, CH * 2 * NIDX16], mybir.dt.int16)
            for ch in range(CH):
                nc.sync.dma_start(
                    out=idxt[:, ch * 2 * NIDX16 : (ch + 1) * 2 * NIDX16],
                    in_=idx_d[:, ch * 2 * NIDX16 : (ch + 1) * 2 * NIDX16],
                )
            browt = idxp.tile([128, F], mybir.dt.float8e4)
            nc.sync.dma_start(out=browt[:], in_=brow_d[:, :])
            w32t = idxp.tile([128, 128], mybir.dt.float8e4)
            nc.sync.dma_start(out=w32t[:], in_=w32_d[:, :])

            low_reg = nc.gpsimd.to_reg(LOW_CNT)
            high_reg = nc.gpsimd.to_reg(HIGH_CNT)

            for ch in range(CH):
                wblk = wblkp.tile([128, 2 * RUNS * 128], mybir.dt.float8e4, tag="wblk")
                nc.sync.dma_start(out=wblk[:], in_=w_d[ch])

                hid = hidp.tile([128, 2 * F], mybir.dt.bfloat16)

                for s in range(2):
                    ibase = (ch * 2 + s) * NIDX16
                    rt = gathp.tile([128, RUNS * F], mybir.dt.float8e4, tag="gath")
                    nc.gpsimd.dma_gather(
                        out_ap=rt[:, : LOW_RUNS * F].rearrange("p (s f) -> p s f", f=F),
                        in_ap=t_low,
                        idxs_ap=idxt[:, ibase : ibase + LOW_CNT // 16],
                        num_idxs=LOW_CNT,
                        num_idxs_reg=low_reg,
                        elem_size=F,
                        single_packet=False,
                    )
                    nc.gpsimd.dma_gather(
                        out_ap=rt[:, LOW_RUNS * F :].rearrange("p (s f) -> p s f", f=F),
                        in_ap=t_high,
                        idxs_ap=idxt[:, ibase + LOW_CNT // 16 : ibase + NIDX16],
                        num_idxs=HIGH_CNT,
                        num_idxs_reg=high_reg,
                        elem_size=F,
                        single_packet=False,
                    )

                    ps = psump.tile([128, F], mybir.dt.float32, tag="ps")
                    for q in range(RUNS // 2):
                        wq = wblk[:, (s * RUNS + 2 * q) * 128 : (s * RUNS + 2 * q + 2) * 128]
                        nc.tensor.matmul(
                            out=ps[:],
                            lhsT=wq.rearrange("p (two f) -> p two f", two=2),
                            rhs=rt[:, 2 * q * F : (2 * q + 2) * F].rearrange(
                                "p (two f) -> p two f", two=2
                            ),
                            start=(q == 0),
                            stop=False,
                            perf_mode=mybir.MatmulPerfMode.DoubleRow,
                        )
                    # RUNS is odd: last data run plays solo
                    nc.tensor.matmul(
                        out=ps[:],
                        lhsT=wblk[:, (s * RUNS + RUNS - 1) * 128 : (s * RUNS + RUNS) * 128],
                        rhs=rt[:, (RUNS - 1) * F : RUNS * F],
                        start=False,
                        stop=False,
                    )
                    nc.tensor.matmul(
                        out=ps[:],
                        lhsT=w32t[:],
                        rhs=browt[:],
                        start=False,
                        stop=True,
                    )

                    nc.vector.tensor_scalar(
                        out=hid[:, s * F : (s + 1) * F],
                        in0=ps[:],
                        scalar1=0.0,
                        scalar2=1.0,
                        op0=mybir.AluOpType.max,
                        op1=mybir.AluOpType.min,
                    )

                wsel = wselp.tile([128, 2 * F], mybir.dt.bfloat16)
                nc.sync.dma_start(out=wsel[:], in_=wsel_d[ch])
                obsel = finp.tile([128, 1], mybir.dt.float32, tag="ob")
                nc.sync.dma_start(out=obsel[:], in_=obsel_d[ch])

                prod = finp.tile([128, 2 * F], mybir.dt.float32, tag="prod")
                nc.vector.tensor_tensor(
                    out=prod[:], in0=hid[:], in1=wsel[:], op=mybir.AluOpType.mult
                )
                acc = finp.tile([128, 1], mybir.dt.float32, tag="acc")
                nc.vector.tensor_reduce(
                    out=acc[:], in_=prod[:], axis=mybir.AxisListType.X, op=mybir.AluOpType.add
                )
                sig = finp.tile([128, 1], mybir.dt.float32, tag="sig")
                nc.scalar.activation(
                    out=sig[:],
                    in_=acc[:],
                    func=mybir.ActivationFunctionType.Sigmoid,
                    bias=obsel[:],
                )
                nc.sync.dma_start(out=out_d[ch * 128 : (ch + 1) * 128, :], in_=sig[:])

    nc.compile()
    return nc


def _get_compiled():
    global _compiled
    if _compiled is None:
        _compiled = _build()
    return _compiled


def _wrap16(lst):
    """int16 index list -> [128, len/16] wrapped (entry j -> [j%16, j//16]),
    replicated to all 128 partitions."""
    n = lst.shape[0]
    w = lst.reshape(n // 16, 16).T.astype(np.int16)
    return np.tile(w, (8, 1))


def _prep_set(idx_core, values_core):
    """Per-core prep for one index set.

    idx_core: [BC, K] int32, values_core: [BC, K] f32
    Returns idx16 [CH, 128, NIDX16] int16, W [CH, 128, RUNS*128] f32.
    """
    idx16 = np.zeros((CH, 128, NIDX16), np.int16)
    W = np.zeros((CH, 128, RUNS, 128), np.float32)
    for ch in range(CH):
        v = idx_core[ch * 128 : (ch + 1) * 128].reshape(-1)        # [4096]
        val = values_core[ch * 128 : (ch + 1) * 128].reshape(-1)
        m_of = np.repeat(np.arange(128), K)

        # dedup rows: each unique row gets ONE gather slot; duplicate lookups
        # merge into the routing matrix (weights sum per (slot, m)).
        vu, inv = np.unique(v, return_inverse=True)
        U = vu.shape[0]
        assert U <= LOW_CNT + HIGH_CNT, U

        strict_low = vu < HSTART
        flex = (~strict_low) & (vu < 32768)
        need = LOW_CNT - int(strict_low.sum())
        assert 0 <= need <= int(flex.sum()), (need, int(flex.sum()))
        flex_rank = np.cumsum(flex) - 1
        is_low = strict_low | (flex & (flex_rank < need))
        nh = U - LOW_CNT
        assert int(is_low.sum()) == LOW_CNT and 0 < nh <= HIGH_CNT

        # slot of each unique row: lows 0..LOW_CNT-1, highs LOW_CNT..U-1
        slot_of = np.empty(U, np.int64)
        slot_of[is_low] = np.arange(LOW_CNT)
        slot_of[~is_low] = LOW_CNT + np.arange(nh)

        ilist = np.zeros(LOW_CNT + HIGH_CNT, np.int16)  # pad = high row 0, w=0
        ilist[:LOW_CNT][np.arange(LOW_CNT)] = vu[is_low]
        ilist[LOW_CNT : LOW_CNT + nh] = vu[~is_low] - HSTART
        idx16[ch, :, : LOW_CNT // 16] = _wrap16(ilist[:LOW_CNT])
        idx16[ch, :, LOW_CNT // 16 :] = _wrap16(ilist[LOW_CNT:])

        s_occ = slot_of[inv]                  # slot of each of the 4096 lookups
        np.add.at(W[ch], (s_occ % 128, s_occ // 128, m_of), val)
    return idx16, W.reshape(CH, 128, RUNS * 128)


def _prep_core(core, T8, values, stm, nstm, wsel_all, obsel_all):
    rows = slice(core * BC, (core + 1) * BC)
    v_core = values[rows]
    idx_stm, w_stm = _prep_set(stm[rows], v_core)
    idx_nstm, w_nstm = _prep_set(nstm[rows], v_core)

    # idx SBUF layout: [128, CH * 2 * NIDX16], chunk-major then set
    idx = np.empty((128, CH * 2 * NIDX16), np.int16)
    for ch in range(CH):
        idx[:, (ch * 2) * NIDX16 : (ch * 2 + 1) * NIDX16] = idx_stm[ch]
        idx[:, (ch * 2 + 1) * NIDX16 : (ch * 2 + 2) * NIDX16] = idx_nstm[ch]

    w_rt = np.concatenate([w_stm, w_nstm], axis=2).astype(FP8)  # [CH,128,2*RUNS*128]

    return {
        "t_tab": T8,
        "idx": idx,
        "w_rt": w_rt,
        "w_sel": wsel_all[rows].reshape(CH, 128, 2 * F).astype(BF16),
        "ob_sel": obsel_all[rows].reshape(CH, 128, 1).astype(np.float32),
    }


def build_in_maps(values, stm_indices, nstm_indices, buckets, ft_w, ft_b, fft_w, fft_b, out_w, out_b):
    values = np.asarray(values, dtype=np.float32)
    stm_indices = np.asarray(stm_indices, dtype=np.int32)
    nstm_indices = np.asarray(nstm_indices, dtype=np.int32)
    buckets = np.asarray(buckets, dtype=np.int32)
    ft_w = np.asarray(ft_w, dtype=np.float32)
    ft_b = np.asarray(ft_b, dtype=np.float32)
    fft_w = np.asarray(fft_w, dtype=np.float32)
    fft_b = np.asarray(fft_b, dtype=np.float32)
    out_w = np.asarray(out_w, dtype=np.float32)
    out_b = np.asarray(out_b, dtype=np.float32)

    T8 = np.concatenate([
        ft_w + np.tile(fft_w, (FT_VOCAB // FFT_VOCAB, 1)),
        np.zeros((1, F), np.float32),
    ]).astype(FP8)                                # [FT_VOCAB + 1, F]
    brow = np.zeros((128, F), np.float32)
    brow[0] = ft_b + fft_b
    brow = brow.astype(FP8)
    w32 = np.zeros((128, 128), np.float32)
    w32[0, :] = 1.0
    w32 = w32.astype(FP8)
    wsel_all = out_w[buckets]                     # [B, 1024] f32
    obsel_all = out_b[buckets]                    # [B] f32

    maps = [
        _prep_core(c, T8, values, stm_indices, nstm_indices, wsel_all, obsel_all)
        for c in range(N_CORES)
    ]
    for m in maps:
        m["bias_row"] = brow
        m["w32"] = w32
    return maps


def kernel(**inputs):
    nc = _get_compiled()
    in_maps = build_in_maps(**inputs)
    res = run_bass_kernel_spmd(nc, in_maps, core_ids=list(range(N_CORES)))
    out = np.concatenate([res.results[c]["out"] for c in range(N_CORES)], axis=0)
    return out.astype(np.float32)
